# revision 20
# baseline (speedup 1.0000x reference)
"""Multi-head attention on 8 TRN2 NeuronCores.

Problem: x[2, 2048, 1024], w_qkv[1024, 3072], w_out[1024, 1024] (f32).
  qkv = x @ w_qkv; q,k,v per 16 heads of dim 64; softmax(q k^T / 8) v; out proj.

Sharding: 16 heads split 8 ways (one head-PAIR per core, both batches on
every core).  Each core computes q^T/k^T/v for its 2 heads over all
B*L = 4096 rows, runs attention, then one 8-rank AllToAll exchanges
(head-pair -> (batch, L/4-chunk)) so each core finishes the output
projection for its own 512 output rows with all 16 heads present.

Layout trick: scores are computed TRANSPOSED (S^T[m, l] tiles) so softmax's
sum runs over the partition axis -- done for free by appending a ones-column
to v in the attn@v matmul (out rows = [o^T; colsums]).  exp() runs on the
Scalar engine straight out of PSUM with the 1/8 scale folded in (scores are
~N(0,1) so no max-subtraction is needed for fp32/bf16 stability).
Normalization happens on small [128, 65] transposed tiles (per-partition
scalars), then transposes back -- total transpose traffic is ~0.5M elements
instead of the 16.7M a P-transpose approach would need.

Compute dtype bf16 (f32 accumulation in PSUM).
"""

import sys

sys.path.insert(0, "/opt/trn_rl_repo")

import numpy as np
import ml_dtypes

import concourse.bass as bass
import concourse.mybir as mybir
import concourse.tile as tile
from concourse import bacc
from concourse import bass_utils
from concourse.masks import make_identity

B, L, D, H, DH = 2, 2048, 1024, 16, 64
BL = B * L  # 4096
SCALE = DH ** -0.5
N_CORES = 8
BF16 = mybir.dt.bfloat16
F32 = mybir.dt.float32
Exp = mybir.ActivationFunctionType.Exp

KT = D // 128          # 8 k-tiles over the model dim
MT = L // 128          # 16 m-tiles per batch
LC = L // 512          # 4 l-chunks of 512 per batch
VT = BL // 128         # 32 v row-tiles over (b, l)


def _build():
    nc = bacc.Bacc("TRN2", target_bir_lowering=False, debug=False,
                   num_devices=N_CORES)
    xT_ext = nc.declare_dram_parameter("xT", [D, BL], BF16, isOutput=False)
    wqk_ext = nc.declare_dram_parameter("wqk", [D, 256], BF16, isOutput=False)
    wv_ext = nc.declare_dram_parameter("wv", [D, 128], BF16, isOutput=False)
    wout_ext = nc.declare_dram_parameter("wout", [D, D], BF16, isOutput=False)
    out_ext = nc.declare_dram_parameter("out", [512, D], F32, isOutput=True)

    with tile.TileContext(nc) as tc:
        with (
            tc.tile_pool(name="big", bufs=1) as big,
            tc.tile_pool(name="pt", bufs=3) as ptp,
            tc.tile_pool(name="small", bufs=3) as small,
            tc.tile_pool(name="psum_st", bufs=3, space="PSUM") as pst,
            tc.tile_pool(name="psum_ov", bufs=1, space="PSUM") as pov,
            tc.tile_pool(name="psum_tr", bufs=1, space="PSUM") as ptr,
            tc.tile_pool(name="dram", bufs=1, space="DRAM") as dram,
        ):
            # ---- static SBUF tensors ----
            xT_t = [big.tile([128, BL], BF16, tag=f"xT{k}", name=f"xT{k}") for k in range(KT)]
            wqk_t = [big.tile([128, 256], BF16, tag=f"wqk{k}", name=f"wqk{k}") for k in range(KT)]
            wv_t = [big.tile([128, 128], BF16, tag=f"wv{k}", name=f"wv{k}") for k in range(KT)]
            for k in range(KT):
                nc.sync.dma_start(
                    xT_t[k][:, 0:512], xT_ext[k * 128:(k + 1) * 128, 0:512])
                nc.sync.dma_start(wqk_t[k][:], wqk_ext[k * 128:(k + 1) * 128, :])
                nc.sync.dma_start(wv_t[k][:], wv_ext[k * 128:(k + 1) * 128, :])
            for cc in range(1, 8):
                for k in range(KT):
                    nc.sync.dma_start(
                        xT_t[k][:, cc * 512:(cc + 1) * 512],
                        xT_ext[k * 128:(k + 1) * 128, cc * 512:(cc + 1) * 512])

            ident_b = big.tile([128, 128], BF16, tag="ident_b")
            make_identity(nc, ident_b[:])
            ident_f = big.tile([128, 128], F32, tag="ident_f")
            make_identity(nc, ident_f[:])

            # q^T and k^T per head, rows 0:64 = head dims, rows 64:128 = 0.
            # Zero-padding keeps the full PE array active so the HAM clock
            # gate stays at 2.4 GHz (half-array matmuls throttle to 1.2).
            qp_t = [[big.tile([128, BL], BF16, tag=f"qp{m}{h}", name=f"qp{m}{h}")
                     for h in range(2)] for m in range(2)]
            for m in range(2):
                for h in range(2):
                    nc.gpsimd.memset(qp_t[m][h][64:128, :], 0.0)
            # v: cols [h*128 : h*128+64] = head h, +64 = ones, rest zero
            v_t = [big.tile([128, 256], BF16, tag=f"v{t}", name=f"v{t}") for t in range(VT)]
            # final o^T for our 2 heads, all 4096 rows
            oT_f = big.tile([128, BL], BF16, tag="oT")

            # ---- QKV projection, one batch at a time ----
            def emit_qkv_cols(ncols, vts):
                for ncol in ncols:
                    for m in range(2):  # 0 -> q, 1 -> k
                        ps = pov.tile([128, 512], F32, tag="ov",
                                      name=f"qk_ps{ncol}_{m}")
                        for k in range(KT):
                            nc.tensor.matmul(
                                ps[:],
                                wqk_t[k][:, m * 128:(m + 1) * 128],
                                xT_t[k][:, ncol * 512:(ncol + 1) * 512],
                                start=(k == 0), stop=(k == KT - 1),
                            )
                        for h in range(2):
                            nc.vector.tensor_copy(
                                qp_t[m][h][0:64, ncol * 512:(ncol + 1) * 512],
                                ps[h * 64:(h + 1) * 64, :])
                for t in vts:
                    ps = ptr.tile([128, 128], F32, tag="tr",
                                  name=f"v_ps{t}")
                    for k in range(KT):
                        nc.tensor.matmul(
                            ps[:],
                            xT_t[k][:, t * 128:(t + 1) * 128],
                            wv_t[k][:],
                            start=(k == 0), stop=(k == KT - 1),
                        )
                    vv = v_t[t][:].rearrange("p (h c) -> p h c", h=2)
                    nc.gpsimd.memset(vv[:, :, 65:128], 0.0)
                    nc.vector.tensor_copy(
                        vv[:, :, 0:64],
                        ps[:].rearrange("p (h c) -> p h c", h=2))
                    nc.gpsimd.memset(vv[:, :, 64:65], 1.0)

            # ---- attention, one (batch, head) unit at a time ----
            # hl outermost: after all hl=0 units, half of oT_f (rows 0:64)
            # is final and its AllToAll overlaps the hl=1 attention.
            cc_in = [dram.tile([N_CORES, 64, 512], BF16, name=f"cc_in{i}")
                     for i in range(2)]
            cc_out = [dram.tile([N_CORES, 64, 512], BF16, name=f"cc_out{i}")
                      for i in range(2)]
            ogT_t = [big.tile([128, 512], BF16, tag=f"wqk{k}", name=f"ogT{k}")
                     for k in range(KT)]
            def emit_attn_unit(hl, b):
                    hs = slice(hl * 64, (hl + 1) * 64)
                    for lc in range(LC):
                        ls = slice(b * L + lc * 512, b * L + (lc + 1) * 512)
                        pt = ptp.tile([128, MT, 512], BF16, tag="pt")
                        ov = pov.tile([128, 512], F32, tag="ov")
                        # S^T pair + exp -> P^T, then immediately the two
                        # attn@v accumulation matmuls for that pair: keeps
                        # ScalarE fed instead of starving it during a
                        # 16-matmul attn@v block.
                        for mp in range(MT // 2):
                            st = pst.tile([128, 1024], F32, tag="st")
                            for h2 in range(2):
                                mt = 2 * mp + h2
                                nc.tensor.matmul(
                                    st[:, h2 * 512:(h2 + 1) * 512],
                                    qp_t[1][hl][:, b * L + mt * 128:
                                                b * L + (mt + 1) * 128],
                                    qp_t[0][hl][:, ls],
                                    start=True, stop=True,
                                )
                            nc.scalar.activation(
                                pt[:, 2 * mp:2 * mp + 2, :], st[:],
                                Exp, scale=SCALE)
                        for mt in range(MT):
                            nc.tensor.matmul(
                                ov[:],
                                v_t[b * MT + mt][:, hl * 128:(hl + 1) * 128],
                                pt[:, mt, :],
                                start=(mt == 0), stop=(mt == MT - 1),
                            )
                        ovs = small.tile([128, 512], F32, tag="ovs")
                        nc.vector.tensor_copy(ovs[:], ov[:])
                        # normalize via small transposes
                        if True:
                            for j in range(4):
                                tr = ptr.tile([128, 128], F32, tag="tr")
                                nc.tensor.transpose(
                                    tr[:], ovs[:, j * 128:(j + 1) * 128],
                                    ident_f[:])
                                rcp = small.tile([128, 1], F32, tag="rcp")
                                nc.vector.reciprocal(rcp[:], tr[:, 64:65])
                                onat = small.tile([128, 64], BF16, tag="onat")
                                nc.vector.tensor_scalar_mul(
                                    onat[:], tr[:, 0:64], rcp[:])
                                tr2 = pov.tile([64, 128], BF16, tag="ov")
                                nc.tensor.transpose(
                                    tr2[:], onat[:], ident_b[:])
                                nc.vector.tensor_copy(
                                    oT_f[hs, b * L + lc * 512 + j * 128:
                                         b * L + lc * 512 + (j + 1) * 128],
                                    tr2[:])

            def emit_a2a(hl):
                # half AllToAll: rows hl*64:(hl+1)*64 of oT_f are final
                hs = slice(hl * 64, (hl + 1) * 64)
                for j in range(N_CORES):
                    nc.sync.dma_start(cc_in[hl][j],
                                      oT_f[hs, j * 512:(j + 1) * 512])
                nc.gpsimd.collective_compute(
                    "AllToAll",
                    mybir.AluOpType.bypass,
                    ins=[cc_in[hl].opt()],
                    outs=[cc_out[hl].opt()],
                    replica_groups=[list(range(N_CORES))],
                )
                for k in range(KT):
                    nc.sync.dma_start(ogT_t[k][hs, :], cc_out[hl][k])

            # interleave: batch-1 qkv fills PE gaps of the first
            # (ScalarE-heavy) attention unit; each half-A2A overlaps
            # the next attention units.
            emit_qkv_cols(range(0, 4), range(0, MT))
            emit_qkv_cols(range(4, 8), range(MT, 2 * MT))
            emit_attn_unit(0, 0)
            emit_attn_unit(0, 1)
            emit_a2a(0)
            emit_attn_unit(1, 0)
            emit_attn_unit(1, 1)
            emit_a2a(1)

            # ---- output projection for our 512 rows ----
            wout_t = [big.tile([128, D], BF16, tag=f"xT{k}", name=f"wout{k}") for k in range(KT)]
            for k in range(KT):
                nc.sync.dma_start(wout_t[k][:], wout_ext[k * 128:(k + 1) * 128, :])
            for lt in range(4):
                for nt in range(2):
                    ps = pst.tile([128, 512], F32, tag="st")
                    for k in range(KT):
                        nc.tensor.matmul(
                            ps[:],
                            ogT_t[k][:, lt * 128:(lt + 1) * 128],
                            wout_t[k][:, nt * 512:(nt + 1) * 512],
                            start=(k == 0), stop=(k == KT - 1),
                        )
                    osb = small.tile([128, 512], F32, tag="osb")
                    nc.vector.tensor_copy(osb[:], ps[:])
                    nc.sync.dma_start(
                        out_ext[lt * 128:(lt + 1) * 128,
                                nt * 512:(nt + 1) * 512],
                        osb[:])

    nc.compile()
    return nc


_NC_CACHE = None


def _get_nc():
    global _NC_CACHE
    if _NC_CACHE is None:
        _NC_CACHE = _build()
    return _NC_CACHE


def _make_in_maps(x, w_qkv, w_out):
    x = np.asarray(x, dtype=np.float32)
    w_qkv = np.asarray(w_qkv, dtype=np.float32)
    w_out = np.asarray(w_out, dtype=np.float32)
    bf = ml_dtypes.bfloat16
    xT = np.ascontiguousarray(
        x.transpose(2, 0, 1).reshape(D, BL)).astype(bf)
    wout_b = w_out.astype(bf)
    in_maps = []
    for c in range(N_CORES):
        cs = slice(c * 128, (c + 1) * 128)
        wqk_c = np.ascontiguousarray(
            np.concatenate([w_qkv[:, cs], w_qkv[:, D:][:, cs]], axis=1)
        ).astype(bf)
        wv_c = np.ascontiguousarray(w_qkv[:, 2 * D:][:, cs]).astype(bf)
        in_maps.append({"xT": xT, "wqk": wqk_c, "wv": wv_c, "wout": wout_b})
    return in_maps


def _run(x, w_qkv, w_out, trace=False):
    nc = _get_nc()
    in_maps = _make_in_maps(x, w_qkv, w_out)
    res = bass_utils.run_bass_kernel_spmd(
        nc, in_maps, list(range(N_CORES)), trace=trace)
    out = np.empty((B, L, D), dtype=np.float32)
    for c in range(N_CORES):
        out[c // 4, (c % 4) * 512:(c % 4 + 1) * 512, :] = \
            np.asarray(res.results[c]["out"])
    return out, res


def kernel(x, w_qkv, w_out):
    out, _ = _run(x, w_qkv, w_out, trace=False)
    return out


# revision 21
# speedup vs baseline: 1.0323x; 1.0323x over previous
"""Multi-head attention on 8 TRN2 NeuronCores.

Problem: x[2, 2048, 1024], w_qkv[1024, 3072], w_out[1024, 1024] (f32).
  qkv = x @ w_qkv; q,k,v per 16 heads of dim 64; softmax(q k^T / 8) v; out proj.

Sharding: 16 heads split 8 ways (one head-PAIR per core, both batches on
every core).  Each core computes q^T/k^T/v for its 2 heads over all
B*L = 4096 rows, runs attention, then one 8-rank AllToAll exchanges
(head-pair -> (batch, L/4-chunk)) so each core finishes the output
projection for its own 512 output rows with all 16 heads present.

Layout trick: scores are computed TRANSPOSED (S^T[m, l] tiles) so softmax's
sum runs over the partition axis -- done for free by appending a ones-column
to v in the attn@v matmul (out rows = [o^T; colsums]).  exp() runs on the
Scalar engine straight out of PSUM with the 1/8 scale folded in (scores are
~N(0,1) so no max-subtraction is needed for fp32/bf16 stability).
Normalization happens on small [128, 65] transposed tiles (per-partition
scalars), then transposes back -- total transpose traffic is ~0.5M elements
instead of the 16.7M a P-transpose approach would need.

Compute dtype bf16 (f32 accumulation in PSUM).
"""

import sys

sys.path.insert(0, "/opt/trn_rl_repo")

import numpy as np
import ml_dtypes

import concourse.bass as bass
import concourse.mybir as mybir
import concourse.tile as tile
from concourse import bacc
from concourse import bass_utils
from concourse.masks import make_identity

B, L, D, H, DH = 2, 2048, 1024, 16, 64
BL = B * L  # 4096
SCALE = DH ** -0.5
N_CORES = 8
BF16 = mybir.dt.bfloat16
F32 = mybir.dt.float32
Exp = mybir.ActivationFunctionType.Exp

KT = D // 128          # 8 k-tiles over the model dim
MT = L // 128          # 16 m-tiles per batch
LC = L // 512          # 4 l-chunks of 512 per batch
VT = BL // 128         # 32 v row-tiles over (b, l)


def _build():
    nc = bacc.Bacc("TRN2", target_bir_lowering=False, debug=False,
                   num_devices=N_CORES)
    xT_ext = nc.declare_dram_parameter("xT", [D, BL], BF16, isOutput=False)
    wqk_ext = nc.declare_dram_parameter("wqk", [D, 256], BF16, isOutput=False)
    wv_ext = nc.declare_dram_parameter("wv", [D, 128], BF16, isOutput=False)
    wout_ext = nc.declare_dram_parameter("wout", [D, D], BF16, isOutput=False)
    out_ext = nc.declare_dram_parameter("out", [512, D], F32, isOutput=True)

    with tile.TileContext(nc) as tc:
        with (
            tc.tile_pool(name="big", bufs=1) as big,
            tc.tile_pool(name="pt", bufs=3) as ptp,
            tc.tile_pool(name="small", bufs=3) as small,
            tc.tile_pool(name="psum_st", bufs=2, space="PSUM") as pst,
            tc.tile_pool(name="psum_ov", bufs=2, space="PSUM") as pov,
            tc.tile_pool(name="psum_tr", bufs=2, space="PSUM") as ptr,
            tc.tile_pool(name="dram", bufs=1, space="DRAM") as dram,
        ):
            # ---- static SBUF tensors ----
            xT_t = [big.tile([128, BL], BF16, tag=f"xT{k}", name=f"xT{k}") for k in range(KT)]
            wqk_t = [big.tile([128, 256], BF16, tag=f"wqk{k}", name=f"wqk{k}") for k in range(KT)]
            wv_t = [big.tile([128, 128], BF16, tag=f"wv{k}", name=f"wv{k}") for k in range(KT)]
            for k in range(KT):
                nc.sync.dma_start(
                    xT_t[k][:, 0:512], xT_ext[k * 128:(k + 1) * 128, 0:512])
                nc.sync.dma_start(wqk_t[k][:], wqk_ext[k * 128:(k + 1) * 128, :])
                nc.sync.dma_start(wv_t[k][:], wv_ext[k * 128:(k + 1) * 128, :])
            for cc in range(1, 8):
                for k in range(KT):
                    nc.sync.dma_start(
                        xT_t[k][:, cc * 512:(cc + 1) * 512],
                        xT_ext[k * 128:(k + 1) * 128, cc * 512:(cc + 1) * 512])

            ident_b = big.tile([128, 128], BF16, tag="ident_b")
            make_identity(nc, ident_b[:])
            ident_f = big.tile([128, 128], F32, tag="ident_f")
            make_identity(nc, ident_f[:])

            # q^T and k^T per head, rows 0:64 = head dims, rows 64:128 = 0.
            # Zero-padding keeps the full PE array active so the HAM clock
            # gate stays at 2.4 GHz (half-array matmuls throttle to 1.2).
            qp_t = [[big.tile([128, BL], BF16, tag=f"qp{m}{h}", name=f"qp{m}{h}")
                     for h in range(2)] for m in range(2)]
            for m in range(2):
                for h in range(2):
                    nc.gpsimd.memset(qp_t[m][h][64:128, :], 0.0)
            # v: cols [h*128 : h*128+64] = head h, +64 = ones, rest zero
            v_t = [big.tile([128, 256], BF16, tag=f"v{t}", name=f"v{t}") for t in range(VT)]
            # final o^T for our 2 heads, all 4096 rows
            oT_f = big.tile([128, BL], BF16, tag="oT")

            # ---- QKV projection, one batch at a time ----
            def emit_qkv_cols(ncols, vts):
                for ncol in ncols:
                    for m in range(2):  # 0 -> q, 1 -> k
                        ps = pov.tile([128, 512], F32, tag="ov",
                                      name=f"qk_ps{ncol}_{m}")
                        for k in range(KT):
                            nc.tensor.matmul(
                                ps[:],
                                wqk_t[k][:, m * 128:(m + 1) * 128],
                                xT_t[k][:, ncol * 512:(ncol + 1) * 512],
                                start=(k == 0), stop=(k == KT - 1),
                            )
                        for h in range(2):
                            nc.vector.tensor_copy(
                                qp_t[m][h][0:64, ncol * 512:(ncol + 1) * 512],
                                ps[h * 64:(h + 1) * 64, :])
                for t in vts:
                    ps = ptr.tile([128, 128], F32, tag="tr",
                                  name=f"v_ps{t}")
                    for k in range(KT):
                        nc.tensor.matmul(
                            ps[:],
                            xT_t[k][:, t * 128:(t + 1) * 128],
                            wv_t[k][:],
                            start=(k == 0), stop=(k == KT - 1),
                        )
                    vv = v_t[t][:].rearrange("p (h c) -> p h c", h=2)
                    nc.gpsimd.memset(vv[:, :, 65:128], 0.0)
                    nc.vector.tensor_copy(
                        vv[:, :, 0:64],
                        ps[:].rearrange("p (h c) -> p h c", h=2))
                    nc.gpsimd.memset(vv[:, :, 64:65], 1.0)

            # ---- attention, one (batch, head) unit at a time ----
            # hl outermost: after all hl=0 units, half of oT_f (rows 0:64)
            # is final and its AllToAll overlaps the hl=1 attention.
            cc_in = [dram.tile([N_CORES, 64, 512], BF16, name=f"cc_in{i}")
                     for i in range(2)]
            cc_out = [dram.tile([N_CORES, 64, 512], BF16, name=f"cc_out{i}")
                      for i in range(2)]
            ogT_t = [big.tile([128, 512], BF16, tag=f"wqk{k}", name=f"ogT{k}")
                     for k in range(KT)]
            def emit_attn_unit(hl, b):
                    hs = slice(hl * 64, (hl + 1) * 64)
                    for lc in range(LC):
                        ls = slice(b * L + lc * 512, b * L + (lc + 1) * 512)
                        pt = ptp.tile([128, MT, 512], BF16, tag="pt")
                        ov = pov.tile([128, 512], F32, tag="ov")
                        # S^T pair + exp -> P^T, then immediately the two
                        # attn@v accumulation matmuls for that pair: keeps
                        # ScalarE fed instead of starving it during a
                        # 16-matmul attn@v block.
                        for mp in range(MT // 2):
                            st = pst.tile([128, 1024], F32, tag="st")
                            for h2 in range(2):
                                mt = 2 * mp + h2
                                nc.tensor.matmul(
                                    st[:, h2 * 512:(h2 + 1) * 512],
                                    qp_t[1][hl][:, b * L + mt * 128:
                                                b * L + (mt + 1) * 128],
                                    qp_t[0][hl][:, ls],
                                    start=True, stop=True,
                                )
                            nc.scalar.activation(
                                pt[:, 2 * mp:2 * mp + 2, :], st[:],
                                Exp, scale=SCALE)
                        for mt in range(MT):
                            nc.tensor.matmul(
                                ov[:],
                                v_t[b * MT + mt][:, hl * 128:(hl + 1) * 128],
                                pt[:, mt, :],
                                start=(mt == 0), stop=(mt == MT - 1),
                            )
                        ovs = small.tile([128, 512], F32, tag="ovs")
                        nc.vector.tensor_copy(ovs[:], ov[:])
                        # normalize via small transposes
                        if True:
                            for j in range(4):
                                tr = ptr.tile([128, 128], F32, tag="tr")
                                nc.tensor.transpose(
                                    tr[:], ovs[:, j * 128:(j + 1) * 128],
                                    ident_f[:])
                                rcp = small.tile([128, 1], F32, tag="rcp")
                                nc.vector.reciprocal(rcp[:], tr[:, 64:65])
                                onat = small.tile([128, 64], BF16, tag="onat")
                                nc.vector.tensor_scalar_mul(
                                    onat[:], tr[:, 0:64], rcp[:])
                                tr2 = pov.tile([64, 128], BF16, tag="ov")
                                nc.tensor.transpose(
                                    tr2[:], onat[:], ident_b[:])
                                nc.vector.tensor_copy(
                                    oT_f[hs, b * L + lc * 512 + j * 128:
                                         b * L + lc * 512 + (j + 1) * 128],
                                    tr2[:])

            def emit_a2a(hl):
                # half AllToAll: rows hl*64:(hl+1)*64 of oT_f are final
                hs = slice(hl * 64, (hl + 1) * 64)
                for j in range(N_CORES):
                    nc.sync.dma_start(cc_in[hl][j],
                                      oT_f[hs, j * 512:(j + 1) * 512])
                nc.gpsimd.collective_compute(
                    "AllToAll",
                    mybir.AluOpType.bypass,
                    ins=[cc_in[hl].opt()],
                    outs=[cc_out[hl].opt()],
                    replica_groups=[list(range(N_CORES))],
                )
                for k in range(KT):
                    nc.sync.dma_start(ogT_t[k][hs, :], cc_out[hl][k])

            # interleave: batch-1 qkv fills PE gaps of the first
            # (ScalarE-heavy) attention unit; each half-A2A overlaps
            # the next attention units.
            emit_qkv_cols(range(0, 4), range(0, MT))
            emit_qkv_cols(range(4, 8), range(MT, 2 * MT))
            emit_attn_unit(0, 0)
            emit_attn_unit(0, 1)
            emit_a2a(0)
            emit_attn_unit(1, 0)
            emit_attn_unit(1, 1)
            emit_a2a(1)

            # ---- output projection for our 512 rows ----
            wout_t = [big.tile([128, D], BF16, tag=f"xT{k}", name=f"wout{k}") for k in range(KT)]
            for k in range(KT):
                nc.sync.dma_start(wout_t[k][:], wout_ext[k * 128:(k + 1) * 128, :])
            for lt in range(4):
                for nt in range(2):
                    ps = pst.tile([128, 512], F32, tag="st")
                    for k in range(KT):
                        nc.tensor.matmul(
                            ps[:],
                            ogT_t[k][:, lt * 128:(lt + 1) * 128],
                            wout_t[k][:, nt * 512:(nt + 1) * 512],
                            start=(k == 0), stop=(k == KT - 1),
                        )
                    osb = small.tile([128, 512], F32, tag="osb")
                    nc.vector.tensor_copy(osb[:], ps[:])
                    nc.sync.dma_start(
                        out_ext[lt * 128:(lt + 1) * 128,
                                nt * 512:(nt + 1) * 512],
                        osb[:])

    nc.compile()
    return nc


_NC_CACHE = None


def _get_nc():
    global _NC_CACHE
    if _NC_CACHE is None:
        _NC_CACHE = _build()
    return _NC_CACHE


def _make_in_maps(x, w_qkv, w_out):
    x = np.asarray(x, dtype=np.float32)
    w_qkv = np.asarray(w_qkv, dtype=np.float32)
    w_out = np.asarray(w_out, dtype=np.float32)
    bf = ml_dtypes.bfloat16
    xT = np.ascontiguousarray(
        x.transpose(2, 0, 1).reshape(D, BL)).astype(bf)
    wout_b = w_out.astype(bf)
    in_maps = []
    for c in range(N_CORES):
        cs = slice(c * 128, (c + 1) * 128)
        wqk_c = np.ascontiguousarray(
            np.concatenate([w_qkv[:, cs], w_qkv[:, D:][:, cs]], axis=1)
        ).astype(bf)
        wv_c = np.ascontiguousarray(w_qkv[:, 2 * D:][:, cs]).astype(bf)
        in_maps.append({"xT": xT, "wqk": wqk_c, "wv": wv_c, "wout": wout_b})
    return in_maps


def _run(x, w_qkv, w_out, trace=False):
    nc = _get_nc()
    in_maps = _make_in_maps(x, w_qkv, w_out)
    res = bass_utils.run_bass_kernel_spmd(
        nc, in_maps, list(range(N_CORES)), trace=trace)
    out = np.empty((B, L, D), dtype=np.float32)
    for c in range(N_CORES):
        out[c // 4, (c % 4) * 512:(c % 4 + 1) * 512, :] = \
            np.asarray(res.results[c]["out"])
    return out, res


def kernel(x, w_qkv, w_out):
    out, _ = _run(x, w_qkv, w_out, trace=False)
    return out
